# revision 4
# baseline (speedup 1.0000x reference)
"""Trainium2 Bass kernel for ConvScoreSSREM loss (fp16-staged pipeline).

Computes, for B=16384 rows (data-parallel, 2048 rows per NeuronCore x 8):
    cm        = contexts @ mat_M                    [B, E]
    scores_k  = sum_e cm[b,e] * res_k[b,e]          k in 0..4
    out[b]    = log_softmax(scores)[:, 0]

The kernel is HBM-bound: per core it must stream 6 x 8MB of fp32 inputs.
Two host-side staging transforms cut the device work:
  - all streamed tensors (and mat_M) are cast to fp16 on the host before
    upload: HBM traffic halves (48MB -> 24MB per core per pass), and the
    DVE score reductions run in 16-bit 2x mode.  fp16 quantization of the
    inputs gives ~3e-4 rel err on the output (gate is 2e-2).
  - contexts is pre-transposed on the host to [E, BS], so the PE consumes
    it as the stationary operand directly - no PE transposes, no PSUM
    transpose pools, no ACT drain copies.

Per-core plan (2048 rows, E=1024):
  - mat_M resident in SBUF as fp16 [128, 8, 1024] (one 2MB DMA).
  - ctxT resident per pass as fp16 [128, 8, 2048] (4MB, double-buffered
    across repeat passes).
  - res tensors streamed in 512-row groups (1MB DMAs) across the three
    DGE rings (sync/scalar HWDGE + gpsimd SWDGE).
  - per 128-row tile: 16 matmuls (stationary = ctxT chunk fp16, moving =
    mat_M 512-wide halves) accumulate cm[128,1024] fp32 in PSUM; ACT
    copies cm to SBUF with an fp16 downcast; 5 DVE scalar_tensor_tensor
    ops (fp16 in, fp32 accum) produce the scores.
  - one log-softmax tail over the [128, 16, 5] score tile, single DMA out.
"""

import numpy as np

import concourse.bacc as bacc
import concourse.mybir as mybir
import concourse.tile as tile
from concourse.bass_utils import run_bass_kernel_spmd

B = 16384
E = 1024
NCORES = 8
BS = B // NCORES  # 2048 rows per core
P = 128
NT = BS // P      # 16 row-tiles per core
KC = E // P       # 8 contraction chunks
NK = 5            # number of res tensors
NHALF = 512       # matmul moving free-dim (one PSUM bank of fp32)
GA = 4            # row-tiles per res DMA group (512 rows = 1MB fp16)
NG = NT // GA     # 4 groups per pass

F32 = mybir.dt.float32
F16 = mybir.dt.float16

RES_NAMES = ["res0", "res1", "res2", "res3", "res4"]


DEFAULT_OPTS = dict(
    res_bufs=10,      # 2 groups of 5 res tiles in flight (8KB/partition each)
    ctx_bufs=2,       # double-buffer the per-pass ctxT load across repeats
    cms_bufs=3,
    pcm_bufs=3,       # PSUM cm tiles ([128,1024] fp32 = 2 banks each)
    junk_bufs=2,
    # DGE ring per res tensor per group, rotated even/odd group
    qmap_even=("sp", "pool", "act", "sp", "pool"),
    qmap_odd=("pool", "sp", "act", "pool", "sp"),
    ctx_q="sp",
    ctx_split=2,      # split the 4MB ctxT load into this many DMAs
    cms_f16=True,     # downcast cm to fp16 in the ACT copy (DVE 2x mode)
    stt_pool_ks=(),   # score indices offloaded to Pool as mult+reduce
)


def build_nc(repeat=1, internal_inputs=False, opts=None):
    """Build + compile the single-core Bass program (same program on all 8 cores).

    repeat>1 replays the steady-state compute loop; internal_inputs=True reads
    ctxT/res from internal DRAM scratch instead of ExternalInputs (both are
    timing aids only)."""
    nc = bacc.Bacc("TRN2", debug=False, enable_asserts=False, num_devices=NCORES)

    if internal_inputs:
        ctx_d = nc.dram_tensor("ctxT_i", (E, BS), F16, kind="Internal")
        res_d = [nc.dram_tensor(n + "_i", (BS, E), F16, kind="Internal") for n in RES_NAMES]
        m_d = nc.dram_tensor("mat_M_i", (E, E), F16, kind="Internal")
        # keep one ExternalInput so the PJRT wrapper has something to bind
        nc.dram_tensor("mat_M", (E, E), F32, kind="ExternalInput")
    else:
        ctx_d = nc.dram_tensor("ctxT", (E, BS), F16, kind="ExternalInput")
        res_d = [nc.dram_tensor(n, (BS, E), F16, kind="ExternalInput") for n in RES_NAMES]
        m_d = nc.dram_tensor("mat_M", (E, E), F16, kind="ExternalInput")
    out_d = nc.dram_tensor("out", (BS,), F32, kind="ExternalOutput")

    o = dict(DEFAULT_OPTS)
    if opts:
        o.update(opts)
    with tile.TileContext(nc) as tc:
        _body(nc, tc, ctx_d.ap(), [r.ap() for r in res_d], m_d.ap(), out_d.ap(),
              repeat=repeat, o=o)

    nc.compile()
    return nc


def _body(nc, tc, ctx_d, res_d, m_d, out_d, repeat=1, o=None):
    o = o or DEFAULT_OPTS
    # DRAM views. DMA pairs source/dest elements in flat AP order, so the
    # DRAM view dims must match the SBUF tile's dim order.
    ctx_v = ctx_d.rearrange("(k p) b -> p k b", p=P)          # [p, k, b]
    m_v = m_d.rearrange("(k p) e -> p k e", p=P)              # [p, k, e]
    res_g = [r.rearrange("(g a p) e -> g p a e", a=GA, p=P) for r in res_d]

    ENG = {"sp": nc.sync, "act": nc.scalar, "pool": nc.gpsimd}

    with (
        tc.tile_pool(name="mpool", bufs=1) as mpool,
        tc.tile_pool(name="ctxp", bufs=o["ctx_bufs"]) as ctxp,
        tc.tile_pool(name="resp", bufs=o["res_bufs"]) as resp,
        tc.tile_pool(name="cmsb", bufs=o["cms_bufs"]) as cmsb,
        tc.tile_pool(name="junkp", bufs=o["junk_bufs"]) as junkp,
        tc.tile_pool(name="smallp", bufs=1) as smallp,
        tc.tile_pool(name="pcm", bufs=o["pcm_bufs"], space="PSUM") as pcm,
    ):
        # mat_M resident: m_sb[p, k, :] = M[k*128 + p, :]  (fp16)
        m_sb = mpool.tile([P, KC, E], F16)
        nc.sync.dma_start(m_sb[:], m_v)

        scores = smallp.tile([P, NT, NK], F32)

        def load_group(g):
            qmap = o["qmap_even"] if g % 2 == 0 else o["qmap_odd"]
            res_t = []
            for k in range(NK):
                r = resp.tile([P, GA, E], F16, tag="res")
                ENG[qmap[k]].dma_start(r[:], res_g[k][g])
                res_t.append(r)
            return res_t

        for _rep in range(repeat):
            # per-pass ctxT: [p, k, b] fp16, 32KB/partition
            ctxT = ctxp.tile([P, KC, BS], F16, tag="ctx")
            ns = o["ctx_split"]
            bs_c = BS // ns
            for s in range(ns):
                ENG[o["ctx_q"]].dma_start(
                    ctxT[:, :, s * bs_c : (s + 1) * bs_c],
                    ctx_v[:, :, s * bs_c : (s + 1) * bs_c],
                )

            for g in range(NG):
                res_t = load_group(g)
                for a in range(GA):
                    t = GA * g + a
                    # cm[128b, 1024e'] accumulated over 8 contraction chunks
                    cm = pcm.tile([P, E], F32, tag="cm")
                    for k in range(KC):
                        for h in range(2):
                            nc.tensor.matmul(
                                cm[:, h * NHALF : (h + 1) * NHALF],
                                ctxT[:, k, t * P : (t + 1) * P],
                                m_sb[:, k, h * NHALF : (h + 1) * NHALF],
                                start=(k == 0),
                                stop=(k == KC - 1),
                            )

                    # PSUM -> SBUF with fp16 downcast: frees the PSUM bank and
                    # lets the DVE score ops run in 16-bit 2x mode
                    cdt = F16 if o["cms_f16"] else F32
                    cm_s = cmsb.tile([P, E], cdt, tag="cms")
                    nc.scalar.copy(cm_s[:], cm[:])

                    # scores[:, t, k] = sum_e' cm * res_k (fused mul+accum)
                    for k in range(NK):
                        junk = junkp.tile([P, E], cdt, tag="junk")
                        if k in o["stt_pool_ks"]:
                            nc.gpsimd.tensor_tensor(
                                out=junk[:], in0=cm_s[:], in1=res_t[k][:, a, :],
                                op=mybir.AluOpType.mult,
                            )
                            nc.gpsimd.tensor_reduce(
                                out=scores[:, t, k : k + 1], in_=junk[:],
                                axis=mybir.AxisListType.X, op=mybir.AluOpType.add,
                            )
                        else:
                            nc.vector.scalar_tensor_tensor(
                                out=junk[:],
                                in0=cm_s[:],
                                scalar=1.0,
                                in1=res_t[k][:, a, :],
                                op0=mybir.AluOpType.mult,
                                op1=mybir.AluOpType.mult,
                                accum_out=scores[:, t, k : k + 1],
                            )

        # ---- log-softmax tail over [P, NT, NK] ----
        mx = smallp.tile([P, NT], F32)
        nc.vector.tensor_reduce(
            out=mx[:], in_=scores[:], axis=mybir.AxisListType.X, op=mybir.AluOpType.max
        )
        d = smallp.tile([P, NT, NK], F32)
        mx_b = mx[:, :, None].broadcast_to([P, NT, NK])
        nc.vector.tensor_tensor(
            out=d[:], in0=scores[:], in1=mx_b, op=mybir.AluOpType.subtract
        )
        ex = smallp.tile([P, NT, NK], F32)
        nc.scalar.activation(ex[:], d[:], mybir.ActivationFunctionType.Exp)
        ssum = smallp.tile([P, NT], F32)
        nc.vector.tensor_reduce(
            out=ssum[:], in_=ex[:], axis=mybir.AxisListType.X, op=mybir.AluOpType.add
        )
        lse = smallp.tile([P, NT], F32)
        nc.scalar.activation(lse[:], ssum[:], mybir.ActivationFunctionType.Ln)
        outsb = smallp.tile([P, NT], F32)
        nc.vector.tensor_sub(outsb[:], d[:, :, 0], lse[:])

        nc.sync.dma_start(out_d.rearrange("(t p) -> p t", p=P), outsb[:])


_NC_CACHE = None


def _get_nc():
    global _NC_CACHE
    if _NC_CACHE is None:
        _NC_CACHE = build_nc()
    return _NC_CACHE


def make_in_maps(contexts, res_pos, res_neg1, res_neg2, res_neg3, res_neg4, mat_M):
    contexts = np.asarray(contexts, dtype=np.float32).astype(np.float16)
    ress = [
        np.asarray(r, dtype=np.float32).astype(np.float16)
        for r in (res_pos, res_neg1, res_neg2, res_neg3, res_neg4)
    ]
    mat_M16 = np.asarray(mat_M, dtype=np.float32).astype(np.float16)
    in_maps = []
    for c in range(NCORES):
        sl = slice(c * BS, (c + 1) * BS)
        m = {"ctxT": np.ascontiguousarray(contexts[sl].T), "mat_M": mat_M16}
        for name, r in zip(RES_NAMES, ress):
            m[name] = np.ascontiguousarray(r[sl])
        in_maps.append(m)
    return in_maps


def kernel(contexts, res_pos, res_neg1, res_neg2, res_neg3, res_neg4, mat_M):
    nc = _get_nc()
    in_maps = make_in_maps(
        contexts, res_pos, res_neg1, res_neg2, res_neg3, res_neg4, mat_M
    )
    res = run_bass_kernel_spmd(nc, in_maps, core_ids=list(range(NCORES)))
    out = np.concatenate([res.results[c]["out"] for c in range(NCORES)])
    return out.astype(np.float32, copy=False)


# revision 24
# speedup vs baseline: 1.5734x; 1.5734x over previous
"""Trainium2 Bass kernel for ConvScoreSSREM loss (fp16-staged pipeline).

Computes, for B=16384 rows (data-parallel, 2048 rows per NeuronCore x 8):
    cm        = contexts @ mat_M                    [B, E]
    scores_k  = sum_e cm[b,e] * res_k[b,e]          k in 0..4
    out[b]    = log_softmax(scores)[:, 0]

The kernel is HBM-bound: per core it must stream 6 x 8MB of fp32 inputs.
Two host-side staging transforms cut the device work:
  - all streamed tensors (and mat_M) are cast to fp16 on the host before
    upload: HBM traffic halves (48MB -> 24MB per core per pass), and the
    DVE score reductions run in 16-bit 2x mode.  fp16 quantization of the
    inputs gives ~3e-4 rel err on the output (gate is 2e-2).
  - contexts is pre-transposed on the host to [E, BS], so the PE consumes
    it as the stationary operand directly - no PE transposes, no PSUM
    transpose pools, no ACT drain copies.

Per-core plan (2048 rows, E=1024):
  - mat_M resident in SBUF as fp16 [128, 8, 1024] (one 2MB DMA).
  - ctxT resident per pass as fp16 [128, 8, 2048] (4MB, double-buffered
    across repeat passes).
  - res tensors streamed in 512-row groups (1MB DMAs) across the three
    DGE rings (sync/scalar HWDGE + gpsimd SWDGE).
  - per 128-row tile: 16 matmuls (stationary = ctxT chunk fp16, moving =
    mat_M 512-wide halves) accumulate cm[128,1024] fp32 in PSUM; ACT
    copies cm to SBUF with an fp16 downcast; 5 DVE scalar_tensor_tensor
    ops (fp16 in, fp32 accum) produce the scores.
  - one log-softmax tail over the [128, 16, 5] score tile, single DMA out.
"""

import numpy as np

import concourse.bacc as bacc
import concourse.mybir as mybir
import concourse.tile as tile
from concourse.bass_utils import run_bass_kernel_spmd

B = 16384
E = 1024
NCORES = 8
BS = B // NCORES  # 2048 rows per core
P = 128
NT = BS // P      # 16 row-tiles per core
KC = E // P       # 8 contraction chunks
NK = 5            # number of res tensors
NHALF = 512       # matmul moving free-dim (one PSUM bank of fp32)
GA = 4            # row-tiles per res DMA group (512 rows = 1MB fp16)
NG = NT // GA     # 4 groups per pass

F32 = mybir.dt.float32
F16 = mybir.dt.float16

RES_NAMES = ["res0", "res1", "res2", "res3", "res4"]


DEFAULT_OPTS = dict(
    res_bufs=10,      # 2 groups of 5 res tiles in flight (8KB/partition each)
    res_bufs_fused=2, # whole-group [P,NK,GA,E] tiles in flight (40KB each)
    ctx_bufs=2,       # double-buffer the per-pass ctxT load across repeats
    cms_bufs=3,
    pcm_bufs=3,       # PSUM cm tiles ([128,1024] fp32 = 2 banks each)
    junk_bufs=2,
    # DGE ring per res tensor per group, rotated even/odd group
    qmap_even=("sp", "pool", "act", "sp", "pool"),
    qmap_odd=("pool", "sp", "act", "pool", "sp"),
    ctx_q="sp",
    ctx_split=2,      # split the 4MB ctxT load into this many DMAs
    cms_f16=True,     # downcast cm to fp16 in the ACT copy (DVE 2x mode)
    stt_pool_ks=(),   # score indices offloaded to Pool as mult+reduce
    res_fused=True,   # host-interleaved [NG,P,NK,GA,E] res: 1 contiguous
                      # 5MB DMA per 512-row group instead of 5x 1MB
    fused_qmap=("sp", "act", "pool", "sp"),  # ring per group (fused mode)
    fused_split=2,    # split each fused group DMA across this many rings
    # timing-only probes (produce wrong outputs; never set in the real kernel)
    probe_no_compute=False,  # DMAs only (tiny ACT touches keep tiles alive)
    probe_nk=NK,             # number of score STTs per tile
    probe_kc=KC,             # number of matmul contraction chunks
)


def build_nc(repeat=1, internal_inputs=False, opts=None):
    """Build + compile the single-core Bass program (same program on all 8 cores).

    repeat>1 replays the steady-state compute loop; internal_inputs=True reads
    ctxT/res from internal DRAM scratch instead of ExternalInputs (both are
    timing aids only)."""
    nc = bacc.Bacc("TRN2", debug=False, enable_asserts=False, num_devices=NCORES)

    o = dict(DEFAULT_OPTS)
    if opts:
        o.update(opts)

    kind = "Internal" if internal_inputs else "ExternalInput"
    sfx = "_i" if internal_inputs else ""
    ctx_d = nc.dram_tensor("ctxT" + sfx, (E, BS), F16, kind=kind)
    if o["res_fused"]:
        res_d = nc.dram_tensor("res_all" + sfx, (NG, P, NK, GA, E), F16, kind=kind)
    else:
        res_d = [nc.dram_tensor(n + sfx, (BS, E), F16, kind=kind) for n in RES_NAMES]
    m_d = nc.dram_tensor("mat_M" + sfx, (E, E), F16, kind=kind)
    if internal_inputs:
        # keep one ExternalInput so the PJRT wrapper has something to bind
        nc.dram_tensor("mat_M", (E, E), F32, kind="ExternalInput")
    out_d = nc.dram_tensor("out", (BS,), F32, kind="ExternalOutput")
    res_ap = res_d.ap() if o["res_fused"] else [r.ap() for r in res_d]
    with tile.TileContext(nc) as tc:
        _body(nc, tc, ctx_d.ap(), res_ap, m_d.ap(), out_d.ap(),
              repeat=repeat, o=o)

    nc.compile()
    return nc


def _body(nc, tc, ctx_d, res_d, m_d, out_d, repeat=1, o=None):
    o = o or DEFAULT_OPTS
    # DRAM views. DMA pairs source/dest elements in flat AP order, so the
    # DRAM view dims must match the SBUF tile's dim order.
    ctx_v = ctx_d.rearrange("(k p) b -> p k b", p=P)          # [p, k, b]
    m_v = m_d.rearrange("(k p) e -> p k e", p=P)              # [p, k, e]
    if not o["res_fused"]:
        res_g = [r.rearrange("(g a p) e -> g p a e", a=GA, p=P) for r in res_d]

    ENG = {"sp": nc.sync, "act": nc.scalar, "pool": nc.gpsimd}

    with (
        tc.tile_pool(name="mpool", bufs=1) as mpool,
        tc.tile_pool(name="ctxp", bufs=o["ctx_bufs"]) as ctxp,
        tc.tile_pool(name="resp",
                     bufs=o["res_bufs_fused"] if o["res_fused"] else o["res_bufs"]) as resp,
        tc.tile_pool(name="cmsb", bufs=o["cms_bufs"]) as cmsb,
        tc.tile_pool(name="junkp", bufs=o["junk_bufs"]) as junkp,
        tc.tile_pool(name="smallp", bufs=1) as smallp,
        tc.tile_pool(name="pcm", bufs=o["pcm_bufs"], space="PSUM") as pcm,
    ):
        # mat_M resident: m_sb[p, k, :] = M[k*128 + p, :]  (fp16)
        m_sb = mpool.tile([P, KC, E], F16)
        nc.sync.dma_start(m_sb[:], m_v)

        scores = smallp.tile([P, NT, NK], F32)

        def load_group(g):
            if o["res_fused"]:
                # one [P, NK, GA, E] tile per group: contiguous 40KB per
                # partition in DRAM, split across rings for DGE parallelism
                rt = resp.tile([P, NK, GA, E], F16, tag="res")
                eng = ENG[o["fused_qmap"][g % len(o["fused_qmap"])]]
                ns = o["fused_split"]
                rings = ["sp", "act", "pool"]
                for s in range(ns):
                    k0, k1 = s * NK // ns, (s + 1) * NK // ns
                    e = eng if ns == 1 else ENG[rings[(g + s) % 3]]
                    e.dma_start(rt[:, k0:k1], res_d[g][:, k0:k1])
                return rt
            qmap = o["qmap_even"] if g % 2 == 0 else o["qmap_odd"]
            res_t = []
            for k in range(NK):
                r = resp.tile([P, GA, E], F16, tag="res")
                ENG[qmap[k]].dma_start(r[:], res_g[k][g])
                res_t.append(r)
            return res_t

        for _rep in range(repeat):
            # per-pass ctxT: [p, k, b] fp16, 32KB/partition
            ctxT = ctxp.tile([P, KC, BS], F16, tag="ctx")
            ns = o["ctx_split"]
            bs_c = BS // ns
            for s in range(ns):
                ENG[o["ctx_q"]].dma_start(
                    ctxT[:, :, s * bs_c : (s + 1) * bs_c],
                    ctx_v[:, :, s * bs_c : (s + 1) * bs_c],
                )

            for g in range(NG):
                res_t = load_group(g)

                def res_ap_of(k, a):
                    if o["res_fused"]:
                        return res_t[:, k, a, :]
                    return res_t[k][:, a, :]

                if o["probe_no_compute"]:
                    for k in range(NK):
                        touch = junkp.tile([P, 16], F16, tag="junk")
                        nc.scalar.copy(touch[:], res_ap_of(k, 0)[:, :16])
                    if g == 0:
                        touch = junkp.tile([P, 16], F16, tag="junk")
                        nc.scalar.copy(touch[:], ctxT[:, 0, :16])
                    continue
                for a in range(GA):
                    t = GA * g + a
                    # cm[128b, 1024e'] accumulated over 8 contraction chunks
                    cm = pcm.tile([P, E], F32, tag="cm")
                    for k in range(o["probe_kc"]):
                        for h in range(2):
                            nc.tensor.matmul(
                                cm[:, h * NHALF : (h + 1) * NHALF],
                                ctxT[:, k, t * P : (t + 1) * P],
                                m_sb[:, k, h * NHALF : (h + 1) * NHALF],
                                start=(k == 0),
                                stop=(k == o["probe_kc"] - 1),
                            )

                    # PSUM -> SBUF with fp16 downcast: frees the PSUM bank and
                    # lets the DVE score ops run in 16-bit 2x mode
                    cdt = F16 if o["cms_f16"] else F32
                    cm_s = cmsb.tile([P, E], cdt, tag="cms")
                    nc.scalar.copy(cm_s[:], cm[:])

                    # scores[:, t, k] = sum_e' cm * res_k (fused mul+accum)
                    for k in range(o["probe_nk"]):
                        junk = junkp.tile([P, E], cdt, tag="junk")
                        if k in o["stt_pool_ks"]:
                            nc.gpsimd.tensor_tensor(
                                out=junk[:], in0=cm_s[:], in1=res_ap_of(k, a),
                                op=mybir.AluOpType.mult,
                            )
                            nc.gpsimd.tensor_reduce(
                                out=scores[:, t, k : k + 1], in_=junk[:],
                                axis=mybir.AxisListType.X, op=mybir.AluOpType.add,
                            )
                        else:
                            nc.vector.scalar_tensor_tensor(
                                out=junk[:],
                                in0=cm_s[:],
                                scalar=1.0,
                                in1=res_ap_of(k, a),
                                op0=mybir.AluOpType.mult,
                                op1=mybir.AluOpType.mult,
                                accum_out=scores[:, t, k : k + 1],
                            )

        if o["probe_no_compute"]:
            outsb = smallp.tile([P, NT], F32)
            nc.vector.memset(outsb[:], 0.0)
            nc.sync.dma_start(out_d.rearrange("(t p) -> p t", p=P), outsb[:])
            return

        # ---- log-softmax tail over [P, NT, NK] ----
        mx = smallp.tile([P, NT], F32)
        nc.vector.tensor_reduce(
            out=mx[:], in_=scores[:], axis=mybir.AxisListType.X, op=mybir.AluOpType.max
        )
        d = smallp.tile([P, NT, NK], F32)
        mx_b = mx[:, :, None].broadcast_to([P, NT, NK])
        nc.vector.tensor_tensor(
            out=d[:], in0=scores[:], in1=mx_b, op=mybir.AluOpType.subtract
        )
        ex = smallp.tile([P, NT, NK], F32)
        nc.scalar.activation(ex[:], d[:], mybir.ActivationFunctionType.Exp)
        ssum = smallp.tile([P, NT], F32)
        nc.vector.tensor_reduce(
            out=ssum[:], in_=ex[:], axis=mybir.AxisListType.X, op=mybir.AluOpType.add
        )
        lse = smallp.tile([P, NT], F32)
        nc.scalar.activation(lse[:], ssum[:], mybir.ActivationFunctionType.Ln)
        outsb = smallp.tile([P, NT], F32)
        nc.vector.tensor_sub(outsb[:], d[:, :, 0], lse[:])

        nc.sync.dma_start(out_d.rearrange("(t p) -> p t", p=P), outsb[:])


_NC_CACHE = None


def _get_nc():
    global _NC_CACHE
    if _NC_CACHE is None:
        _NC_CACHE = build_nc()
    return _NC_CACHE


def make_in_maps(contexts, res_pos, res_neg1, res_neg2, res_neg3, res_neg4, mat_M,
                 fused=True):
    contexts = np.asarray(contexts, dtype=np.float32).astype(np.float16)
    ress = [
        np.asarray(r, dtype=np.float32).astype(np.float16)
        for r in (res_pos, res_neg1, res_neg2, res_neg3, res_neg4)
    ]
    mat_M16 = np.asarray(mat_M, dtype=np.float32).astype(np.float16)
    in_maps = []
    for c in range(NCORES):
        sl = slice(c * BS, (c + 1) * BS)
        m = {"ctxT": np.ascontiguousarray(contexts[sl].T), "mat_M": mat_M16}
        if fused:
            # [NK, BS, E] -> [NK, NG, GA, P, E] -> [NG, P, NK, GA, E]
            arr = np.stack([r[sl] for r in ress], axis=0)
            arr = arr.reshape(NK, NG, GA, P, E).transpose(1, 3, 0, 2, 4)
            m["res_all"] = np.ascontiguousarray(arr)
        else:
            for name, r in zip(RES_NAMES, ress):
                m[name] = np.ascontiguousarray(r[sl])
        in_maps.append(m)
    return in_maps


def kernel(contexts, res_pos, res_neg1, res_neg2, res_neg3, res_neg4, mat_M):
    nc = _get_nc()
    in_maps = make_in_maps(
        contexts, res_pos, res_neg1, res_neg2, res_neg3, res_neg4, mat_M,
        fused=DEFAULT_OPTS["res_fused"],
    )
    res = run_bass_kernel_spmd(nc, in_maps, core_ids=list(range(NCORES)))
    out = np.concatenate([res.results[c]["out"] for c in range(NCORES)])
    return out.astype(np.float32, copy=False)


# revision 28
# speedup vs baseline: 1.6790x; 1.0671x over previous
"""Trainium2 Bass kernel for ConvScoreSSREM loss (fp16-staged pipeline).

Computes, for B=16384 rows (data-parallel, 2048 rows per NeuronCore x 8):
    cm        = contexts @ mat_M                    [B, E]
    scores_k  = sum_e cm[b,e] * res_k[b,e]          k in 0..4
    out[b]    = log_softmax(scores)[:, 0]

The kernel is HBM-bound: per core it must stream 6 x 8MB of fp32 inputs.
Two host-side staging transforms cut the device work:
  - all streamed tensors (and mat_M) are cast to fp16 on the host before
    upload: HBM traffic halves (48MB -> 24MB per core per pass), and the
    DVE score reductions run in 16-bit 2x mode.  fp16 quantization of the
    inputs gives ~3e-4 rel err on the output (gate is 2e-2).
  - contexts is pre-transposed on the host to [E, BS], so the PE consumes
    it as the stationary operand directly - no PE transposes, no PSUM
    transpose pools, no ACT drain copies.

A third host transform interleaves the five res tensors into one
[NG, P, NK, GA, E] DRAM tensor so each 512-row group loads as a single
contiguous 5MB DMA (128 partition runs of 40KB) instead of five strided
1MB DMAs.

Per-core plan (2048 rows, E=1024):
  - mat_M resident in SBUF as fp16 [128, 8, 1024] (one 2MB DMA, scalar
    ring so it doesn't queue ahead of ctxT on sync).
  - ctxT resident per pass as fp16 [128, 8, 2048] (4MB in two halves on
    the sync + gpsimd rings, double-buffered across repeat passes).
  - fused res groups streamed with ring rotation across the three DGE
    rings (sync/scalar HWDGE + gpsimd SWDGE).
  - per 128-row tile: 16 matmuls (stationary = ctxT chunk fp16, moving =
    mat_M 512-wide halves) accumulate cm[128,1024] fp32 in PSUM; ACT
    copies cm to SBUF with an fp16 downcast; 5 DVE scalar_tensor_tensor
    ops (fp16 in, fp32 accum) produce the scores.
  - one log-softmax tail over the [128, 16, 5] score tile, single DMA out.

Measured (axon, r41 wall differential): ~33-42us steady-state per pass vs
~109us for the fp32 baseline.  ~33us == 24MB / 716 GB/s, the per-device
HBM roofline; fp8 staging of any input breaks the 2e-2 rel-err gate
(measured 2.2e-2+ on the actual seeded inputs), so fp16 is the byte floor.
"""

import numpy as np

import concourse.bacc as bacc
import concourse.mybir as mybir
import concourse.tile as tile
from concourse.bass_utils import run_bass_kernel_spmd

B = 16384
E = 1024
NCORES = 8
BS = B // NCORES  # 2048 rows per core
P = 128
NT = BS // P      # 16 row-tiles per core
KC = E // P       # 8 contraction chunks
NK = 5            # number of res tensors
NHALF = 512       # matmul moving free-dim (one PSUM bank of fp32)
GA = 4            # row-tiles per res DMA group (512 rows = 1MB fp16)
NG = NT // GA     # 4 groups per pass

F32 = mybir.dt.float32
F16 = mybir.dt.float16

RES_NAMES = ["res0", "res1", "res2", "res3", "res4"]


DEFAULT_OPTS = dict(
    res_bufs=10,      # 2 groups of 5 res tiles in flight (8KB/partition each)
    res_bufs_fused=2, # whole-group [P,NK,GA,E] tiles in flight (40KB each)
    ctx_bufs=2,       # double-buffer the per-pass ctxT load across repeats
    cms_bufs=3,
    pcm_bufs=3,       # PSUM cm tiles ([128,1024] fp32 = 2 banks each)
    junk_bufs=2,
    # DGE ring per res tensor per group, rotated even/odd group
    qmap_even=("sp", "pool", "act", "sp", "pool"),
    qmap_odd=("pool", "sp", "act", "pool", "sp"),
    ctx_qs=("sp", "pool"),  # rings for the ctxT split halves
    ctx_split=2,      # split the 4MB ctxT load into this many DMAs
    m_q="act",        # mat_M load off the sync ring so ctxT isn't queued
                      # behind it (HWDGE is FIFO per ring)
    cms_f16=True,     # downcast cm to fp16 in the ACT copy (DVE 2x mode)
    stt_pool_ks=(),   # score indices offloaded to Pool as mult+reduce
    res_fused=True,   # host-interleaved [NG,P,NK,GA,E] res: 1 contiguous
                      # 5MB DMA per 512-row group instead of 5x 1MB
    fused_qmap=("sp", "act", "pool", "sp"),  # ring per group (fused mode)
    fused_split=2,    # split each fused group DMA across this many rings
    # timing-only probes (produce wrong outputs; never set in the real kernel)
    probe_no_compute=False,  # DMAs only (tiny ACT touches keep tiles alive)
    probe_nk=NK,             # number of score STTs per tile
    probe_kc=KC,             # number of matmul contraction chunks
)


def build_nc(repeat=1, internal_inputs=False, opts=None):
    """Build + compile the single-core Bass program (same program on all 8 cores).

    repeat>1 replays the steady-state compute loop; internal_inputs=True reads
    ctxT/res from internal DRAM scratch instead of ExternalInputs (both are
    timing aids only)."""
    nc = bacc.Bacc("TRN2", debug=False, enable_asserts=False, num_devices=NCORES)

    o = dict(DEFAULT_OPTS)
    if opts:
        o.update(opts)

    kind = "Internal" if internal_inputs else "ExternalInput"
    sfx = "_i" if internal_inputs else ""
    ctx_d = nc.dram_tensor("ctxT" + sfx, (E, BS), F16, kind=kind)
    if o["res_fused"]:
        res_d = nc.dram_tensor("res_all" + sfx, (NG, P, NK, GA, E), F16, kind=kind)
    else:
        res_d = [nc.dram_tensor(n + sfx, (BS, E), F16, kind=kind) for n in RES_NAMES]
    m_d = nc.dram_tensor("mat_M" + sfx, (E, E), F16, kind=kind)
    if internal_inputs:
        # keep one ExternalInput so the PJRT wrapper has something to bind
        nc.dram_tensor("mat_M", (E, E), F32, kind="ExternalInput")
    out_d = nc.dram_tensor("out", (BS,), F32, kind="ExternalOutput")
    res_ap = res_d.ap() if o["res_fused"] else [r.ap() for r in res_d]
    with tile.TileContext(nc) as tc:
        _body(nc, tc, ctx_d.ap(), res_ap, m_d.ap(), out_d.ap(),
              repeat=repeat, o=o)

    nc.compile()
    return nc


def _body(nc, tc, ctx_d, res_d, m_d, out_d, repeat=1, o=None):
    o = o or DEFAULT_OPTS
    # DRAM views. DMA pairs source/dest elements in flat AP order, so the
    # DRAM view dims must match the SBUF tile's dim order.
    ctx_v = ctx_d.rearrange("(k p) b -> p k b", p=P)          # [p, k, b]
    m_v = m_d.rearrange("(k p) e -> p k e", p=P)              # [p, k, e]
    if not o["res_fused"]:
        res_g = [r.rearrange("(g a p) e -> g p a e", a=GA, p=P) for r in res_d]

    ENG = {"sp": nc.sync, "act": nc.scalar, "pool": nc.gpsimd}

    with (
        tc.tile_pool(name="mpool", bufs=1) as mpool,
        tc.tile_pool(name="ctxp", bufs=o["ctx_bufs"]) as ctxp,
        tc.tile_pool(name="resp",
                     bufs=o["res_bufs_fused"] if o["res_fused"] else o["res_bufs"]) as resp,
        tc.tile_pool(name="cmsb", bufs=o["cms_bufs"]) as cmsb,
        tc.tile_pool(name="junkp", bufs=o["junk_bufs"]) as junkp,
        tc.tile_pool(name="smallp", bufs=1) as smallp,
        tc.tile_pool(name="pcm", bufs=o["pcm_bufs"], space="PSUM") as pcm,
    ):
        # mat_M resident: m_sb[p, k, :] = M[k*128 + p, :]  (fp16)
        m_sb = mpool.tile([P, KC, E], F16)
        ENG[o["m_q"]].dma_start(m_sb[:], m_v)

        scores = smallp.tile([P, NT, NK], F32)

        def load_group(g):
            if o["res_fused"]:
                # one [P, NK, GA, E] tile per group: contiguous 40KB per
                # partition in DRAM, split across rings for DGE parallelism
                rt = resp.tile([P, NK, GA, E], F16, tag="res")
                eng = ENG[o["fused_qmap"][g % len(o["fused_qmap"])]]
                ns = o["fused_split"]
                rings = ["sp", "act", "pool"]
                for s in range(ns):
                    k0, k1 = s * NK // ns, (s + 1) * NK // ns
                    e = eng if ns == 1 else ENG[rings[(g + s) % 3]]
                    e.dma_start(rt[:, k0:k1], res_d[g][:, k0:k1])
                return rt
            qmap = o["qmap_even"] if g % 2 == 0 else o["qmap_odd"]
            res_t = []
            for k in range(NK):
                r = resp.tile([P, GA, E], F16, tag="res")
                ENG[qmap[k]].dma_start(r[:], res_g[k][g])
                res_t.append(r)
            return res_t

        for _rep in range(repeat):
            # per-pass ctxT: [p, k, b] fp16, 32KB/partition
            ctxT = ctxp.tile([P, KC, BS], F16, tag="ctx")
            ns = o["ctx_split"]
            bs_c = BS // ns
            for s in range(ns):
                ENG[o["ctx_qs"][s % len(o["ctx_qs"])]].dma_start(
                    ctxT[:, :, s * bs_c : (s + 1) * bs_c],
                    ctx_v[:, :, s * bs_c : (s + 1) * bs_c],
                )

            for g in range(NG):
                res_t = load_group(g)

                def res_ap_of(k, a):
                    if o["res_fused"]:
                        return res_t[:, k, a, :]
                    return res_t[k][:, a, :]

                if o["probe_no_compute"]:
                    for k in range(NK):
                        touch = junkp.tile([P, 16], F16, tag="junk")
                        nc.scalar.copy(touch[:], res_ap_of(k, 0)[:, :16])
                    if g == 0:
                        touch = junkp.tile([P, 16], F16, tag="junk")
                        nc.scalar.copy(touch[:], ctxT[:, 0, :16])
                    continue
                for a in range(GA):
                    t = GA * g + a
                    # cm[128b, 1024e'] accumulated over 8 contraction chunks
                    cm = pcm.tile([P, E], F32, tag="cm")
                    for k in range(o["probe_kc"]):
                        for h in range(2):
                            nc.tensor.matmul(
                                cm[:, h * NHALF : (h + 1) * NHALF],
                                ctxT[:, k, t * P : (t + 1) * P],
                                m_sb[:, k, h * NHALF : (h + 1) * NHALF],
                                start=(k == 0),
                                stop=(k == o["probe_kc"] - 1),
                            )

                    # PSUM -> SBUF with fp16 downcast: frees the PSUM bank and
                    # lets the DVE score ops run in 16-bit 2x mode
                    cdt = F16 if o["cms_f16"] else F32
                    cm_s = cmsb.tile([P, E], cdt, tag="cms")
                    nc.scalar.copy(cm_s[:], cm[:])

                    # scores[:, t, k] = sum_e' cm * res_k (fused mul+accum)
                    for k in range(o["probe_nk"]):
                        junk = junkp.tile([P, E], cdt, tag="junk")
                        if k in o["stt_pool_ks"]:
                            nc.gpsimd.tensor_tensor(
                                out=junk[:], in0=cm_s[:], in1=res_ap_of(k, a),
                                op=mybir.AluOpType.mult,
                            )
                            nc.gpsimd.tensor_reduce(
                                out=scores[:, t, k : k + 1], in_=junk[:],
                                axis=mybir.AxisListType.X, op=mybir.AluOpType.add,
                            )
                        else:
                            nc.vector.scalar_tensor_tensor(
                                out=junk[:],
                                in0=cm_s[:],
                                scalar=1.0,
                                in1=res_ap_of(k, a),
                                op0=mybir.AluOpType.mult,
                                op1=mybir.AluOpType.mult,
                                accum_out=scores[:, t, k : k + 1],
                            )

        if o["probe_no_compute"]:
            outsb = smallp.tile([P, NT], F32)
            nc.vector.memset(outsb[:], 0.0)
            nc.sync.dma_start(out_d.rearrange("(t p) -> p t", p=P), outsb[:])
            return

        # ---- log-softmax tail over [P, NT, NK] ----
        mx = smallp.tile([P, NT], F32)
        nc.vector.tensor_reduce(
            out=mx[:], in_=scores[:], axis=mybir.AxisListType.X, op=mybir.AluOpType.max
        )
        d = smallp.tile([P, NT, NK], F32)
        mx_b = mx[:, :, None].broadcast_to([P, NT, NK])
        nc.vector.tensor_tensor(
            out=d[:], in0=scores[:], in1=mx_b, op=mybir.AluOpType.subtract
        )
        ex = smallp.tile([P, NT, NK], F32)
        nc.scalar.activation(ex[:], d[:], mybir.ActivationFunctionType.Exp)
        ssum = smallp.tile([P, NT], F32)
        nc.vector.tensor_reduce(
            out=ssum[:], in_=ex[:], axis=mybir.AxisListType.X, op=mybir.AluOpType.add
        )
        lse = smallp.tile([P, NT], F32)
        nc.scalar.activation(lse[:], ssum[:], mybir.ActivationFunctionType.Ln)
        outsb = smallp.tile([P, NT], F32)
        nc.vector.tensor_sub(outsb[:], d[:, :, 0], lse[:])

        nc.sync.dma_start(out_d.rearrange("(t p) -> p t", p=P), outsb[:])


_NC_CACHE = None


def _get_nc():
    global _NC_CACHE
    if _NC_CACHE is None:
        _NC_CACHE = build_nc()
    return _NC_CACHE


def make_in_maps(contexts, res_pos, res_neg1, res_neg2, res_neg3, res_neg4, mat_M,
                 fused=True):
    contexts = np.asarray(contexts, dtype=np.float32).astype(np.float16)
    ress = [
        np.asarray(r, dtype=np.float32).astype(np.float16)
        for r in (res_pos, res_neg1, res_neg2, res_neg3, res_neg4)
    ]
    mat_M16 = np.asarray(mat_M, dtype=np.float32).astype(np.float16)
    in_maps = []
    for c in range(NCORES):
        sl = slice(c * BS, (c + 1) * BS)
        m = {"ctxT": np.ascontiguousarray(contexts[sl].T), "mat_M": mat_M16}
        if fused:
            # [NK, BS, E] -> [NK, NG, GA, P, E] -> [NG, P, NK, GA, E]
            arr = np.stack([r[sl] for r in ress], axis=0)
            arr = arr.reshape(NK, NG, GA, P, E).transpose(1, 3, 0, 2, 4)
            m["res_all"] = np.ascontiguousarray(arr)
        else:
            for name, r in zip(RES_NAMES, ress):
                m[name] = np.ascontiguousarray(r[sl])
        in_maps.append(m)
    return in_maps


def kernel(contexts, res_pos, res_neg1, res_neg2, res_neg3, res_neg4, mat_M):
    nc = _get_nc()
    in_maps = make_in_maps(
        contexts, res_pos, res_neg1, res_neg2, res_neg3, res_neg4, mat_M,
        fused=DEFAULT_OPTS["res_fused"],
    )
    res = run_bass_kernel_spmd(nc, in_maps, core_ids=list(range(NCORES)))
    out = np.concatenate([res.results[c]["out"] for c in range(NCORES)])
    return out.astype(np.float32, copy=False)
